# revision 38
# baseline (speedup 1.0000x reference)
"""MoE FFN (DeepSeek-style top-2 routing + shared expert) on 8 TRN2 cores.

Sharding: expert-parallel for the 8 routed experts (core e owns expert e,
host gathers/pads its top-2 tokens to a fixed capacity C); the shared
expert is split 2 token-halves x 4 F-quarters (384 F-rows each). Host does
router + dispatch/combine; device does all FLOPs-heavy matmuls.

All per-core device I/O is packed into ONE [128, WIN] bf16 input and ONE
[128, WOUT] f32 output: per-call dispatch overhead scales with the NUMBER
of bound I/O tensors (~0.15-0.25 ms each under the axon/PJRT path), not
with bytes, so 10 tensors -> 2 tensors removes ~2 ms of overhead. Each
packed block is laid out exactly as its SBUF tile ([128, w] column slab),
so every DMA is a single contiguous column-range read.

Self-contained: hardcodes B=2,S=2048,D=768,E=8,K=2,F=1536.
"""
import ml_dtypes
import numpy as np
from contextlib import ExitStack

import concourse.bacc as bacc
import concourse.mybir as mybir
import concourse.tile as tile
from concourse.bass_utils import run_bass_kernel_spmd

B, S, D = 2, 2048, 768
E, TOPK, F = 8, 2, 1536
T = B * S
NCORES = 8
KD = D // 128            # 6 contraction chunks over D
MF = F // 128            # 12 f-tiles for routed experts
MD = D // 128            # 6 output d-tiles
FS = 384                 # shared-expert F-slice per core (4 slices x 2 halves)
MFS = FS // 128          # 3 f-tiles for shared slice
TH = T // 2              # shared-expert token half
NT = 512                 # moving-operand (token) tile
CAP = 1024               # fixed per-expert device capacity (= T*TOPK/E); the
                         # few overflow tokens of hot experts run on the host

F32 = mybir.dt.float32
BF16 = mybir.dt.bfloat16
NPBF = ml_dtypes.bfloat16

_cache: dict = {}


def _chunks(total, step=NT):
    out, o = [], 0
    while o < total:
        n = min(step, total - o)
        out.append((o, n))
        o += n
    return out


def _iters(C):
    # shared-expert tiles FIRST (their weights are 8x smaller, so the PE
    # starts early while the big routed-expert weights stream in behind),
    # interleaved with routed tiles; a full tile last shortens the drain.
    s_it = [("S", o, n) for o, n in _chunks(TH)]
    r_it = [("R", o, n) for o, n in _chunks(C)]
    return [s_it[0], s_it[1], r_it[0], s_it[2]] + \
        ([r_it[1]] if len(r_it) > 1 else []) + [s_it[3]] + r_it[2:]


def _layout(C):
    """Column offsets of every block in the packed input / output."""
    iters = _iters(C)
    xoff, ooff = [], []
    xi = oi = 0
    for _, _, n in iters:
        xoff.append(xi)
        xi += KD * n
        ooff.append(oi)
        oi += MD * n
    woff = {}
    for name, width in [("sg", KD * FS), ("su", KD * FS), ("sd", MFS * D),
                        ("wg", KD * F), ("wu", KD * F), ("wd", MF * D)]:
        woff[name] = xi
        xi += width
    return iters, xoff, ooff, woff, xi, oi


def _build(C, reps=1):
    """One SPMD program: routed expert over C tokens + shared slice over TH.

    reps>1 replicates the x-load/compute/store pipeline (weights stay
    resident) — used by test.py to amplify exec time above dispatch noise.
    """
    iters, xoff, ooff, woff, WIN, WOUT = _layout(C)
    NIT = len(iters)
    seq = list(range(NIT)) * reps
    nc = bacc.Bacc("TRN2", debug=False)
    inp = nc.dram_tensor("inp", [128, WIN], BF16, kind="ExternalInput")
    out = nc.dram_tensor("out", [128, WOUT], BF16, kind="ExternalOutput")

    with tile.TileContext(nc) as tc, ExitStack() as ctx:
        wpool = ctx.enter_context(tc.tile_pool(name="w", bufs=1))
        xpool = ctx.enter_context(tc.tile_pool(name="x", bufs=6))
        hpool = ctx.enter_context(tc.tile_pool(name="h", bufs=2))
        spool = ctx.enter_context(tc.tile_pool(name="s", bufs=3))
        lpool = ctx.enter_context(tc.tile_pool(name="l", bufs=6))
        pgp = ctx.enter_context(tc.tile_pool(name="pg", bufs=2, space="PSUM"))
        pup = ctx.enter_context(tc.tile_pool(name="pu", bufs=3, space="PSUM"))
        pyp = ctx.enter_context(tc.tile_pool(name="py", bufs=3, space="PSUM"))

        def load_w(name, width, tag):
            t = wpool.tile([128, width], BF16, tag=tag)
            nc.sync.dma_start(t[:], inp[:, woff[name]:woff[name] + width])
            return t

        def load_x(i):
            _, _, n = iters[i]
            xt = xpool.tile([128, KD * NT], BF16, tag="xt")
            nc.sync.dma_start(xt[:, :KD * n],
                              inp[:, xoff[i]:xoff[i] + KD * n])
            return xt

        # The DMA engines drain transfers in SEQ-issue arrival order, so
        # issue EVERYTHING from the SP sequencer in exact first-need order:
        # startup tiles, then m-major gate/up thirds interleaved so R0's
        # m-tiles stream in just ahead of consumption, wd before R0's mm2.
        wg_sb = wpool.tile([128, KD * F], BF16, tag="wg")
        wu_sb = wpool.tile([128, KD * F], BF16, tag="wu")
        xts = [load_x(0)]
        sg_sb = load_w("sg", KD * FS, "sg")
        su_sb = load_w("su", KD * FS, "su")
        xts.append(load_x(1))
        sd_sb = load_w("sd", MFS * D, "sd")
        xts.append(load_x(2))
        third = KD * F // 3
        for q in range(3):
            nc.sync.dma_start(
                wg_sb[:, q * third:(q + 1) * third],
                inp[:, woff["wg"] + q * third:woff["wg"] + (q + 1) * third])
            nc.sync.dma_start(
                wu_sb[:, q * third:(q + 1) * third],
                inp[:, woff["wu"] + q * third:woff["wu"] + (q + 1) * third])
        xts.append(load_x(3))
        wd_sb = load_w("wd", MF * D, "wd")
        xts.append(load_x(4))
        xts.append(load_x(5))

        def mm1(i, xt):
            """gate/up matmuls + silu/mul -> hT tiles for one iteration."""
            ph, _, n = iters[i]
            # all gate/up weights are m-major: m-tile m depends only on
            # columns [m*768, (m+1)*768) of its weight block
            woffs = lambda k, m: m * KD * 128 + k * 128
            if ph == "R":
                g_w, u_w, mf = wg_sb, wu_sb, MF
            else:
                g_w, u_w, mf = sg_sb, su_sb, MFS
            hT = []
            for m in range(mf):
                g = pgp.tile([128, NT], F32, tag="pg")
                u = pup.tile([128, NT], F32, tag="pu")
                for k in range(KD):
                    nc.tensor.matmul(g[:, :n],
                                     g_w[:, woffs(k, m):woffs(k, m) + 128],
                                     xt[:, k * n:k * n + n],
                                     start=(k == 0), stop=(k == KD - 1))
                for k in range(KD):
                    nc.tensor.matmul(u[:, :n],
                                     u_w[:, woffs(k, m):woffs(k, m) + 128],
                                     xt[:, k * n:k * n + n],
                                     start=(k == 0), stop=(k == KD - 1))
                sil = spool.tile([128, NT], F32, tag="sil")
                nc.scalar.activation(sil[:, :n], g[:, :n],
                                     mybir.ActivationFunctionType.Silu)
                h = hpool.tile([128, NT], BF16, tag=f"h{m}")
                nc.vector.tensor_mul(h[:, :n], sil[:, :n], u[:, :n])
                hT.append(h)
            return hT

        def mm2(i, hT):
            """down-projection; each [128,n] output slab streams out (bf16)
            right after its PSUM->SBUF copy, alternating HWDGE rings."""
            ph, _, n = iters[i]
            d_w = wd_sb if ph == "R" else sd_sb
            mf = MF if ph == "R" else MFS
            for m2 in range(MD):
                y = pyp.tile([128, NT], F32, tag="py")
                for k2 in range(mf):
                    nc.tensor.matmul(y[:, :n],
                                     d_w[:, k2 * D + m2 * 128:k2 * D + m2 * 128 + 128],
                                     hT[k2][:, :n],
                                     start=(k2 == 0), stop=(k2 == mf - 1))
                yl = lpool.tile([128, NT], BF16, tag="yl")
                # copies live on DVE only: on ACT they queue ahead of the
                # silus that free mm1's PSUM banks and stall the PE mid-chain
                nc.vector.tensor_copy(yl[:, :n], y[:, :n])
                eng = nc.scalar if m2 % 2 == 0 else nc.sync
                eng.dma_start(out[:, ooff[i] + m2 * n:ooff[i] + m2 * n + n],
                              yl[:, :n])

        # software pipeline: emit MM1(i+1) before MM2(i) so the PE chews on
        # the next tile's gate/up while ACT/DVE finish hT(i).
        hprev = None
        for p, ip in enumerate(seq):
            if p >= NIT and p % NIT == 0:
                for i in range(NIT):
                    xts.append(load_x(i))
            h = mm1(ip, xts[p])
            if hprev is not None:
                mm2(seq[p - 1], hprev)
            hprev = h
        mm2(seq[-1], hprev)
    nc.compile()
    return nc


def _router(xf, w_router, expert_bias):
    """Replicates the reference router. f64 for stable top-k ordering,
    f32 softmax (same formula as jax.nn.softmax) for the weights."""
    logits = xf.astype(np.float64) @ w_router.T.astype(np.float64)
    l32 = (xf @ w_router.T).astype(np.float32)
    m = l32.max(-1, keepdims=True)
    e32 = np.exp(l32 - m)
    scores = e32 / e32.sum(-1, keepdims=True)
    e64 = np.exp(logits - logits.max(-1, keepdims=True))
    sel = e64 / e64.sum(-1, keepdims=True) + expert_bias.astype(np.float64)[None, :]
    top_idx = np.argsort(-sel, axis=-1, kind="stable")[:, :TOPK]
    top_s = np.take_along_axis(scores, top_idx, axis=-1)
    top_s = top_s / (top_s.sum(-1, keepdims=True) + 1e-9)
    return top_idx, top_s


def _pack_rows(a):
    """(128*nk, w) row-major -> [128, nk*w] with chunk k at cols [k*w,(k+1)*w)."""
    nk = a.shape[0] // 128
    return a.reshape(nk, 128, a.shape[1]).transpose(1, 0, 2).reshape(128, -1)


def _pack_m_major(a):
    """(128*nk, 128*nm) -> [128, nm*nk*128]: block m holds k-chunks of the
    m-th 128-column slice (matches mm1's shared-weight slicing)."""
    nk, nm = a.shape[0] // 128, a.shape[1] // 128
    t = a.reshape(nk, 128, nm, 128)
    return t.transpose(1, 2, 0, 3).reshape(128, nm * nk * 128)


def _prepare(x, w_router, expert_bias, Wg, Wu, Wd, sg, su, sd):
    """Router + host dispatch + packed per-core input maps."""
    x = np.asarray(x)
    xf = x.reshape(-1, D).astype(np.float32)
    top_idx, top_s = _router(xf, np.asarray(w_router), np.asarray(expert_bias))

    idxs, ws, ov_idxs, ov_ws = [], [], [], []
    for e in range(E):
        hit = (top_idx == e)
        tok = np.nonzero(hit.any(-1))[0]
        w = top_s[tok][hit[tok]]
        idxs.append(tok[:CAP])
        ws.append(w[:CAP])
        ov_idxs.append(tok[CAP:])
        ov_ws.append(w[CAP:])
    C = CAP
    iters, xoff, ooff, woff, WIN, WOUT = _layout(C)

    Wg = np.asarray(Wg, dtype=np.float32)
    Wu = np.asarray(Wu, dtype=np.float32)
    Wd = np.asarray(Wd, dtype=np.float32)
    sg = np.asarray(sg, dtype=np.float32)
    su = np.asarray(su, dtype=np.float32)
    sd = np.asarray(sd, dtype=np.float32)

    in_maps = []
    for e in range(E):
        th, fq = e // 4, e % 4
        buf = np.zeros((128, WIN), NPBF)
        xeT = np.zeros((D, C), np.float32)
        xeT[:, :len(idxs[e])] = xf[idxs[e]].T
        xsT = xf[th * TH:(th + 1) * TH].T
        for i, (ph, o, n) in enumerate(iters):
            src = xeT if ph == "R" else xsT
            buf[:, xoff[i]:xoff[i] + KD * n] = _pack_rows(src[:, o:o + n])
        buf[:, woff["sg"]:woff["sg"] + KD * FS] = \
            _pack_m_major(sg[fq * FS:(fq + 1) * FS].T)
        buf[:, woff["su"]:woff["su"] + KD * FS] = \
            _pack_m_major(su[fq * FS:(fq + 1) * FS].T)
        buf[:, woff["sd"]:woff["sd"] + MFS * D] = \
            _pack_rows(np.ascontiguousarray(sd[:, fq * FS:(fq + 1) * FS].T))
        buf[:, woff["wg"]:woff["wg"] + KD * F] = \
            _pack_m_major(np.ascontiguousarray(Wg[e].T))
        buf[:, woff["wu"]:woff["wu"] + KD * F] = \
            _pack_m_major(np.ascontiguousarray(Wu[e].T))
        buf[:, woff["wd"]:woff["wd"] + MF * D] = \
            _pack_rows(np.ascontiguousarray(Wd[e].T))
        in_maps.append({"inp": buf})
    return C, iters, ooff, idxs, ws, ov_idxs, ov_ws, in_maps


def kernel(x, w_router, expert_bias, Wg, Wu, Wd, sg, su, sd):
    x = np.asarray(x)
    C, iters, ooff, idxs, ws, ov_idxs, ov_ws, in_maps = _prepare(
        x, w_router, expert_bias, Wg, Wu, Wd, sg, su, sd)

    if C not in _cache:
        _cache[C] = _build(C)
    nc = _cache[C]

    try:
        res = run_bass_kernel_spmd(nc, in_maps, core_ids=list(range(NCORES)))
    except Exception:
        # transient device errors (e.g. NRT_EXEC_UNIT_UNRECOVERABLE on a
        # wedged core) usually clear on re-dispatch
        res = run_bass_kernel_spmd(nc, in_maps, core_ids=list(range(NCORES)))

    out = np.zeros((T, D), np.float32)
    xf = x.reshape(-1, D).astype(np.float32)
    for e in range(E):
        th = e // 4
        packed = res.results[e]["out"]
        yeT = np.empty((D, C), np.float32)
        zT = np.empty((D, TH), np.float32)
        for i, (ph, o, n) in enumerate(iters):
            blk = packed[:, ooff[i]:ooff[i] + MD * n]
            blk = blk.reshape(128, MD, n).transpose(1, 0, 2).reshape(D, n)
            (yeT if ph == "R" else zT)[:, o:o + n] = blk
        out[idxs[e]] += ws[e][:, None] * yeT.T[:len(idxs[e])]
        out[th * TH:(th + 1) * TH] += zT.T
        if len(ov_idxs[e]):
            # capacity-overflow tokens of hot experts: host f32 SwiGLU
            xo = xf[ov_idxs[e]]
            g = xo @ np.asarray(Wg[e], np.float32).T
            u = xo @ np.asarray(Wu[e], np.float32).T
            h = (g / (1.0 + np.exp(-g))) * u
            out[ov_idxs[e]] += ov_ws[e][:, None] * \
                (h @ np.asarray(Wd[e], np.float32).T)
    return out.reshape(B, S, D).astype(x.dtype)
